# revision 15
# baseline (speedup 1.0000x reference)
"""Trainium2 Bass kernel for nn_Connector_77738908057780 (dense_mlp).

Computation (see reference):
  x   = image_features                      [B, N, H]    bf16
  f1  = mean(hidden[0:13],  axis=0)         [B, N, H]
  f2  = mean(hidden[13:26], axis=0)         [B, N, H]
  cat = concat([x, f1, f2], -1)             [B, N, 3H]
  h   = gelu(cat @ W1.T + b1)               W1 = nf4_dequant(codes1, scales1) [H, 3H]
  fg  = h @ W2.T + b2                       W2 = nf4_dequant(codes2, scales2) [H, H]
  out = w * LN(fg) + (1-w) * LN(x),         w = sigmoid(alpha)

Sharding: data-parallel over batch B=8 -> one batch element per NeuronCore.

Per-core plan (v3 -- chunked pipeline):
  - 6 token chunks of 128 (last chunk overlaps the previous by 39 tokens;
    identical values stored twice -- partial-partition DMA falls off the
    16-engine SDMA path and runs ~15x slower, so all tiles stay full-128).
  - The 26-layer `hidden` stream dominates HBM traffic (46 MB/core); it is
    issued as 12 large 3.8 MB DMAs on the sync HWDGE queue in chunk order so
    DMA stays saturated end-to-end.  Weights stream on the scalar queue
    behind chunk 0's loads.
  - layer sums entirely on DVE (GPSIMD port contention halves DVE throughput
    when both run -- measured), tree-shaped to amortize dispatch and release
    the hid tile early for DMA slot reuse.
  - cat^T is never materialized: GEMM1's k-loop reads x^T (host-transposed
    input), s1^T and s2^T (TensorE identity-transpose -> PSUM -> ACT copy)
    as three separate SBUF tiles.  No SBUF->SBUF xbar DMA at all.
  - GEMM1 weights-stationary -> h^T in PSUM; GELU(+b1 per-partition bias) on
    ACT -> g^T feeds GEMM2 as stationary; b2 is added by a rank-1 matmul
    (ones-row x b2-row) inside the accumulation group; ACT drains PSUM->fg
    while computing sum(fg) via accum_out.
  - LN stats: ACT accum_out gives S(v), S(v^2); DVE combines to mean/var,
    reciprocal+sqrt for rsqrt.  The gate combine uses 4x-mode tensor_scalar
    ops when the folded LN gains are feature-uniform (they are: ln gains are
    ones, biases zeros), falling back to scalar_tensor_tensor otherwise.

NF4 dequant of the (small, replicated) weights is host-side weight prep; the
bf16 weights are less DMA traffic than the int32 codes.
"""

import os
import sys

import numpy as np
import ml_dtypes

for _p in ("/opt/trn_rl_repo", "/root/.axon_site/_ro/trn_rl_repo"):
    if os.path.isdir(_p) and _p not in sys.path:
        sys.path.insert(0, _p)

import concourse.bass as bass
import concourse.mybir as mybir
import concourse.tile as tile
from concourse import bacc
from concourse import bass_utils

BF16 = mybir.dt.bfloat16
F32 = mybir.dt.float32
AF = mybir.ActivationFunctionType
ALU = mybir.AluOpType

NP_BF16 = ml_dtypes.bfloat16

P = 128
H = 1152
H3 = 3456
NT = 729          # tokens per core (N); B=8 cores
L = 26
KO1 = H3 // P     # 27 k-tiles for GEMM1
KO2 = H // P      # 9 k-tiles for GEMM2
MO = H // P       # 9 output-feature tiles
EPS = 1e-5
NCHUNK = 3        # fg free-dim chunks of 384
CH = H // NCHUNK  # 384

# Token chunks; the last starts at 601 so it is a full 128 tokens (tokens
# 601..639 are computed twice with identical values).
CHUNK_STARTS = [0, 128, 256, 384, 512, 601]
NCH = len(CHUNK_STARTS)

NF4_CODEBOOK = np.array([
    -1.0, -0.6961928009986877, -0.5250730514526367, -0.39491748809814453,
    -0.28444138169288635, -0.18477343022823334, -0.09105003625154495, 0.0,
    0.07958029955625534, 0.16093020141124725, 0.24611230194568634,
    0.33791524171829224, 0.4407098591327667, 0.5626170039176941,
    0.7229568362236023, 1.0], dtype=np.float32)

BLOCK = 64


def _dequant_nf4(codes, scales):
    """Match reference: codebook lookup * per-64-block absmax, cast bf16."""
    out_f, in_f = codes.shape
    w = NF4_CODEBOOK[codes].reshape(out_f, in_f // BLOCK, BLOCK)
    w = w * scales[:, :, None].astype(np.float32)
    return w.reshape(out_f, in_f)  # float32 (caller casts)


def _build_program(act=AF.Gelu, uniform_gate=True):
    nc = bacc.Bacc(
        "TRN2",
        target_bir_lowering=False,
        debug=False,
        num_devices=1,
    )
    x_d = nc.dram_tensor("x", (NT, H), BF16, kind="ExternalInput").ap()
    xtc_d = nc.dram_tensor("xtc", (NCH, P, MO, P), BF16, kind="ExternalInput").ap()
    hid_d = nc.dram_tensor("hid", (L, NT, H), BF16, kind="ExternalInput").ap()
    w1t_d = nc.dram_tensor("w1t", (H3, H), BF16, kind="ExternalInput").ap()
    w2t_d = nc.dram_tensor("w2t", (H, H), BF16, kind="ExternalInput").ap()
    b1s_d = nc.dram_tensor("b1s", (P, MO), F32, kind="ExternalInput").ap()
    b2s_d = nc.dram_tensor("b2s", (1, H), BF16, kind="ExternalInput").ap()
    ident_d = nc.dram_tensor("ident", (P, P), BF16, kind="ExternalInput").ap()
    # uniform path: gsc = per-partition (G1, G2) scalars; bcs = (Bc, 0)
    gsc_d = nc.dram_tensor("gsc", (P, 2), F32, kind="ExternalInput").ap()
    bcs_d = nc.dram_tensor("bcs", (P, 2), F32, kind="ExternalInput").ap()
    # general path: per-feature broadcasts
    g1b_d = nc.dram_tensor("g1b", (P, H), BF16, kind="ExternalInput").ap()
    g2b_d = nc.dram_tensor("g2b", (P, H), BF16, kind="ExternalInput").ap()
    bcb_d = nc.dram_tensor("bcb", (P, H), BF16, kind="ExternalInput").ap()
    out_d = nc.dram_tensor("out", (NT, H), BF16, kind="ExternalOutput").ap()

    with tile.TileContext(nc) as tc:
        _program(nc, tc, x_d, xtc_d, hid_d, w1t_d, w2t_d, b1s_d, b2s_d,
                 ident_d, gsc_d, bcs_d, g1b_d, g2b_d, bcb_d, out_d, act,
                 uniform_gate)

    nc.compile()
    return nc


def _program(nc, tc, x_d, xtc_d, hid_d, w1t_d, w2t_d, b1s_d, b2s_d, ident_d,
             gsc_d, bcs_d, g1b_d, g2b_d, bcb_d, out_d, act, uniform_gate):
    with (
        tc.tile_pool(name="consts", bufs=1) as cpool,
        tc.tile_pool(name="hid", bufs=4) as hpool,
        tc.tile_pool(name="xt", bufs=2) as xtpool,
        tc.tile_pool(name="x", bufs=3) as xpool,
        tc.tile_pool(name="scr", bufs=1) as scrpool,
        tc.tile_pool(name="acc", bufs=2) as apool,
        tc.tile_pool(name="st", bufs=2) as stpool,
        tc.tile_pool(name="g", bufs=2) as gpool,
        tc.tile_pool(name="fg", bufs=2) as fgpool,
        tc.tile_pool(name="tmp", bufs=2) as tpool,
        tc.tile_pool(name="dum", bufs=1) as dpool,
        tc.tile_pool(name="stats", bufs=2) as spool,
        tc.tile_pool(name="ps1", bufs=1, space="PSUM") as ps1pool,
        tc.tile_pool(name="ps2", bufs=3, space="PSUM") as ps2pool,
        tc.tile_pool(name="pt", bufs=2, space="PSUM") as ptpool,
    ):
        # ---- small constants first (sync queue; ~50 KB total) ----
        b1s_sb = cpool.tile([P, MO], F32)
        nc.sync.dma_start(b1s_sb, b1s_d)
        b2s_sb = cpool.tile([1, H], BF16)
        nc.sync.dma_start(b2s_sb, b2s_d)
        ident_sb = cpool.tile([P, P], BF16)
        nc.sync.dma_start(ident_sb, ident_d)
        if uniform_gate:
            gsc_sb = cpool.tile([P, 2], F32)
            nc.sync.dma_start(gsc_sb, gsc_d)
            bcs_sb = cpool.tile([P, 2], F32)
            nc.sync.dma_start(bcs_sb, bcs_d)
        else:
            g1b_sb = cpool.tile([P, H], BF16)
            nc.sync.dma_start(g1b_sb, g1b_d)
            g2b_sb = cpool.tile([P, H], BF16)
            nc.sync.dma_start(g2b_sb, g2b_d)
            bcb_sb = cpool.tile([P, H], BF16)
            nc.sync.dma_start(bcb_sb, bcb_d)
        ones_sb = cpool.tile([1, P], BF16)
        nc.vector.memset(ones_sb, 1.0)

        w1t_sb = cpool.tile([P, KO1, H], BF16)
        w2t_sb = cpool.tile([P, KO2, H], BF16)
        w1t_r = w1t_d.rearrange("(ko p) n -> p ko n", p=P)

        dummy = dpool.tile([P, H], BF16, tag="dummy")
        # DVE-serial scratch for the layer-sum trees (reused across chunks)
        scr = [scrpool.tile([P, 3, H], BF16, name=f"scr{i}", tag=f"scr{i}")
               for i in range(2)]

        def half_sum(h7, h6, dst, scr):
            """dst[t, f] = sum over the 7-layer and 6-layer pieces, on DVE.

            Tree-shaped to amortize DVE dispatch; each hid piece is fully
            consumed after two ops so its DMA slot recycles early."""
            t7 = scr[0]
            nc.vector.tensor_add(t7, h7[:, 0:3, :], h7[:, 3:6, :])
            nc.vector.tensor_add(t7[:, 2, :], t7[:, 2, :], h7[:, 6, :])
            t6 = scr[1]
            nc.vector.tensor_add(t6, h6[:, 0:3, :], h6[:, 3:6, :])
            nc.vector.tensor_add(t7, t7, t6)
            nc.vector.tensor_add(dst, t7[:, 0, :], t7[:, 1, :])
            nc.vector.tensor_add(dst, dst, t7[:, 2, :])

        def transpose_to(src, dst):
            """src [P, H] token-major -> dst [P, MO, P] feature-major."""
            for g0 in (0, 4, 8):
                g = min(4, MO - g0)
                pt = ptpool.tile([P, 4, P], BF16, tag="pt")
                for j in range(g):
                    nc.tensor.transpose(
                        pt[:, j, :],
                        src[:, (g0 + j) * P:(g0 + j + 1) * P],
                        ident_sb)
                nc.scalar.activation(dst[:, g0:g0 + g, :],
                                     pt[:, 0:g, :], AF.Copy)

        for c, t0 in enumerate(CHUNK_STARTS):
            # ---- DMA issues (loads only; stores go at the chunk end) ----
            hps = []
            for l0, nl in ((0, 7), (7, 6), (13, 7), (20, 6)):
                hp = hpool.tile([P, 7, H], BF16, tag="hid")
                nc.sync.dma_start(
                    hp[:, 0:nl, :],
                    hid_d[l0:l0 + nl, t0:t0 + P, :].rearrange(
                        "l p f -> p l f"))
                hps.append(hp)
            xt = xtpool.tile([P, MO, P], BF16, tag="xtc")
            nc.scalar.dma_start(xt, xtc_d[c])
            if c == 0:
                # weights stream behind chunk 0's x^T on the scalar queue,
                # ordered so GEMM1's k-outer loop can start early
                nc.scalar.dma_start(w1t_sb[:, 0:9, :], w1t_r[:, 0:9, :])
            xc = xpool.tile([P, H], BF16, tag="x")
            nc.scalar.dma_start(xc, x_d[t0:t0 + P, :])
            if c == 0:
                nc.scalar.dma_start(w1t_sb[:, 9:18, :], w1t_r[:, 9:18, :])
                nc.scalar.dma_start(w1t_sb[:, 18:27, :], w1t_r[:, 18:27, :])
                nc.scalar.dma_start(
                    w2t_sb, w2t_d.rearrange("(ko p) n -> p ko n", p=P))

            # ---- 13-layer sums on DVE ----
            s1 = apool.tile([P, H], BF16, tag="s1")
            half_sum(hps[0], hps[1], s1, scr)
            s2 = apool.tile([P, H], BF16, tag="s2")
            half_sum(hps[2], hps[3], s2, scr)

            sacc = spool.tile([P, 8], F32, tag="sacc")

            # ---- GEMM1 (weights-stationary, k-outer) with s1/s2 transposes
            # (TensorE identity -> PSUM -> ACT copy) issued ahead of the
            # k-groups that consume them, so the ACT copies hide under the
            # preceding matmuls.  The final k-group is m-outer so each GELU
            # fires as soon as its m-tile finishes.
            ps1 = ps1pool.tile([P, MO, P], F32, tag="ps1")
            s1T = stpool.tile([P, MO, P], BF16, tag="s1T")
            s2T = stpool.tile([P, MO, P], BF16, tag="s2T")
            gT = gpool.tile([P, MO, P], BF16, tag="gT")

            def mm1(kk, mm, rhs):
                # start=True marks the whole 2KB PSUM bank pending-zero, so
                # only the first matmul touching each bank sets it; the
                # other m-slices' first writes land on still-pending bytes
                # and overwrite (HW has_written semantics; sim mirrors it).
                nc.tensor.matmul(
                    ps1[:, mm, :],
                    lhsT=w1t_sb[:, kk, mm * P:(mm + 1) * P],
                    rhs=rhs,
                    start=(kk == 0 and mm % 4 == 0),
                    stop=(kk == KO1 - 1),
                    skip_group_check=True,
                )

            transpose_to(s1, s1T)
            for kk in range(0, MO):
                for mm in range(MO):
                    mm1(kk, mm, xt[:, kk, :])
            for kk in range(MO, MO + 5):
                for mm in range(MO):
                    mm1(kk, mm, s1T[:, kk - MO, :])
            transpose_to(s2, s2T)
            for kk in range(MO + 5, 2 * MO):
                for mm in range(MO):
                    mm1(kk, mm, s1T[:, kk - MO, :])
            for mm in range(MO):
                for kk in range(2 * MO, 3 * MO):
                    mm1(kk, mm, s2T[:, kk - 2 * MO, :])
                nc.scalar.activation(gT[:, mm, :], ps1[:, mm, :], act,
                                     bias=b1s_sb[:, mm:mm + 1])

            # ---- LN1(x) raw sums on ACT (fill the GELU->drain gap) ----
            nc.scalar.activation(dummy, xc, AF.Copy,
                                 accum_out=sacc[:, 0:1])
            nc.scalar.activation(dummy, xc, AF.Square,
                                 accum_out=sacc[:, 2:3])

            # ---- GEMM2 (g^T-stationary, k-outer) + b2 rank-1 + ACT drain --
            fg = fgpool.tile([P, H], BF16, tag="fg")
            ps2s = [ps2pool.tile([P, CH], F32, name=f"ps2_{nn}", tag="ps2")
                    for nn in range(NCHUNK)]
            for kk in range(KO2):
                for nn in range(NCHUNK):
                    nc.tensor.matmul(
                        ps2s[nn],
                        lhsT=gT[:, kk, :],
                        rhs=w2t_sb[:, kk, nn * CH:(nn + 1) * CH],
                        start=(kk == 0),
                        stop=False,
                    )
            for nn in range(NCHUNK):
                nc.tensor.matmul(
                    ps2s[nn],
                    lhsT=ones_sb,
                    rhs=b2s_sb[0:1, nn * CH:(nn + 1) * CH],
                    start=False,
                    stop=True,
                )
                nc.scalar.activation(fg[:, nn * CH:(nn + 1) * CH],
                                     ps2s[nn], AF.Copy,
                                     accum_out=sacc[:, 4 + nn:5 + nn])
            nc.scalar.activation(dummy, fg, AF.Square,
                                 accum_out=sacc[:, 3:4])

            # ---- LN stats -> mean / rsqrt(var+eps) for x and fg ----
            deriv = spool.tile([P, 8], F32, tag="deriv")
            nc.vector.tensor_add(sacc[:, 1:2], sacc[:, 4:5], sacc[:, 5:6])
            nc.vector.tensor_add(sacc[:, 1:2], sacc[:, 1:2], sacc[:, 6:7])
            # cols 0,1 = mean(x), mean(fg); 2,3 = E[v^2]+eps; 4,5 = mu^2
            nc.vector.tensor_scalar_mul(deriv[:, 0:2], sacc[:, 0:2], 1.0 / H)
            nc.vector.tensor_scalar(deriv[:, 2:4], sacc[:, 2:4],
                                    1.0 / H, EPS, ALU.mult, ALU.add)
            nc.vector.tensor_tensor(deriv[:, 4:6], deriv[:, 0:2],
                                    deriv[:, 0:2], ALU.mult)
            nc.vector.tensor_tensor(deriv[:, 6:8], deriv[:, 2:4],
                                    deriv[:, 4:6], ALU.subtract)
            igt = spool.tile([P, 2], F32, tag="ig")
            nc.vector.reciprocal(igt, deriv[:, 6:8])
            nc.scalar.activation(igt, igt, AF.Sqrt)

            # ---- normalize + sigmoid gate, store ----
            tmp1 = tpool.tile([P, H], BF16, tag="tmp1")
            if uniform_gate:
                # acol = (G1*ig1, G2*ig2); Bc folded via bcs (always 0 here)
                acol = spool.tile([P, 2], F32, tag="acol")
                nc.vector.tensor_tensor(acol, igt, gsc_sb, ALU.mult)
                # tmp1 = (x - mu1) * a1   (4x-mode tensor_scalar)
                nc.vector.tensor_scalar(tmp1, xc, deriv[:, 0:1],
                                        acol[:, 0:1], ALU.subtract, ALU.mult)
                # fg <- (fg - mu2) * a2   (in place)
                nc.vector.tensor_scalar(fg, fg, deriv[:, 1:2],
                                        acol[:, 1:2], ALU.subtract, ALU.mult)
                # tmp1 <- (tmp1 + Bc) + fg
                nc.vector.scalar_tensor_tensor(
                    tmp1, tmp1, bcs_sb[:, 0:1], fg, ALU.add, ALU.add)
            else:
                # tmp1 = (x - mu1) * G1;  G1 = (1-w)*ln1_g  (broadcast)
                nc.vector.scalar_tensor_tensor(
                    tmp1, xc, deriv[:, 0:1], g1b_sb,
                    ALU.subtract, ALU.mult)
                # fg <- (fg - mu2) * G2;  G2 = w*ln2_g   (in place)
                nc.vector.scalar_tensor_tensor(
                    fg, fg, deriv[:, 1:2], g2b_sb,
                    ALU.subtract, ALU.mult)
                # tmp1 = tmp1 * ig1 + Bc;  Bc = w*ln2_b + (1-w)*ln1_b
                nc.vector.scalar_tensor_tensor(
                    tmp1, tmp1, igt[:, 0:1], bcb_sb,
                    ALU.mult, ALU.add)
                # tmp1 <- fg * ig2 + tmp1   (final output)
                nc.vector.scalar_tensor_tensor(
                    tmp1, fg, igt[:, 1:2], tmp1,
                    ALU.mult, ALU.add)
            nc.scalar.dma_start(out_d[t0:t0 + P, :], tmp1)


_NC_CACHE = {}


def _get_nc(uniform_gate=True):
    key = ("nc", uniform_gate)
    if key not in _NC_CACHE:
        _NC_CACHE[key] = _build_program(uniform_gate=uniform_gate)
    return _NC_CACHE[key]


def _host_prep(codes1, scales1, b1, codes2, scales2, b2,
               ln1_g, ln1_b, ln2_g, ln2_b, alpha):
    # W1 with 1/13 folded into the f1/f2 column blocks (mean -> sum)
    w1 = _dequant_nf4(codes1, scales1)
    # match reference rounding: dequant result is cast to bf16 first
    w1 = w1.astype(NP_BF16).astype(np.float32)
    w1[:, H:] *= np.float32(1.0 / 13.0)
    w1t = np.ascontiguousarray(w1.T).astype(NP_BF16)

    w2 = _dequant_nf4(codes2, scales2).astype(NP_BF16)
    w2t = np.ascontiguousarray(w2.astype(np.float32).T).astype(NP_BF16)

    b1s = np.ascontiguousarray(
        b1.astype(np.float32).reshape(MO, P).T)  # [P, MO]
    b2s = np.ascontiguousarray(b2.astype(NP_BF16).reshape(1, H))

    ident = np.eye(P, dtype=NP_BF16)

    a32 = alpha.astype(np.float32)
    w_gate = (1.0 / (1.0 + np.exp(-a32[0]))).astype(NP_BF16)
    one_minus = (NP_BF16(1.0) - w_gate)
    g1 = (one_minus.astype(np.float32) * ln1_g.astype(np.float32))
    g2 = (w_gate.astype(np.float32) * ln2_g.astype(np.float32))
    bc = (w_gate.astype(np.float32) * ln2_b.astype(np.float32)
          + one_minus.astype(np.float32) * ln1_b.astype(np.float32))

    uniform = (np.ptp(g1) == 0.0 and np.ptp(g2) == 0.0 and np.all(bc == 0.0))
    gsc = np.ascontiguousarray(
        np.broadcast_to(np.array([g1[0], g2[0]], np.float32), (P, 2)))
    bcs = np.zeros((P, 2), np.float32)

    g1b = np.ascontiguousarray(np.broadcast_to(g1.astype(NP_BF16), (P, H)))
    g2b = np.ascontiguousarray(np.broadcast_to(g2.astype(NP_BF16), (P, H)))
    bcb = np.ascontiguousarray(np.broadcast_to(bc.astype(NP_BF16), (P, H)))
    return w1t, w2t, b1s, b2s, ident, gsc, bcs, g1b, g2b, bcb, uniform


def _xtc_prep(x):
    """[729, H] token-major -> [NCH, P, MO, P] feature-major token chunks."""
    xT = np.ascontiguousarray(x.T).reshape(MO, P, NT)
    out = np.empty((NCH, P, MO, P), dtype=NP_BF16)
    for c, t0 in enumerate(CHUNK_STARTS):
        out[c] = xT[:, :, t0:t0 + P].transpose(1, 0, 2)
    return out


def make_in_maps(image_features, hidden, codes1, scales1, b1, codes2, scales2,
                 b2, ln1_g, ln1_b, ln2_g, ln2_b, alpha):
    (w1t, w2t, b1s, b2s, ident, gsc, bcs, g1b, g2b, bcb,
     uniform) = _host_prep(codes1, scales1, b1, codes2, scales2, b2,
                           ln1_g, ln1_b, ln2_g, ln2_b, alpha)
    B = image_features.shape[0]
    in_maps = []
    for c in range(B):
        xc = np.ascontiguousarray(image_features[c]).astype(NP_BF16, copy=False)
        in_maps.append({
            "x": xc,
            "xtc": _xtc_prep(xc),
            "hid": np.ascontiguousarray(hidden[:, c]).astype(NP_BF16, copy=False),
            "w1t": w1t, "w2t": w2t, "b1s": b1s, "b2s": b2s, "ident": ident,
            "gsc": gsc, "bcs": bcs,
            "g1b": g1b, "g2b": g2b, "bcb": bcb,
        })
    return in_maps, uniform


def kernel(image_features, hidden, codes1, scales1, b1, codes2, scales2, b2,
           ln1_g, ln1_b, ln2_g, ln2_b, alpha, _trace=False):
    B, N, Hin = image_features.shape
    assert (B, N, Hin) == (8, NT, H), (B, N, Hin)
    in_maps, uniform = make_in_maps(
        image_features, hidden, codes1, scales1, b1, codes2, scales2, b2,
        ln1_g, ln1_b, ln2_g, ln2_b, alpha)
    nc = _get_nc(uniform_gate=uniform)
    res = bass_utils.run_bass_kernel_spmd(
        nc, in_maps, core_ids=list(range(8)), trace=_trace)
    out = np.stack([res.results[c]["out"] for c in range(8)])
    if _trace:
        kernel._last_results = res
    return out.astype(image_features.dtype, copy=False)


# revision 19
# speedup vs baseline: 1.0272x; 1.0272x over previous
"""Trainium2 Bass kernel for nn_Connector_77738908057780 (dense_mlp).

Computation (see reference):
  x   = image_features                      [B, N, H]    bf16
  f1  = mean(hidden[0:13],  axis=0)         [B, N, H]
  f2  = mean(hidden[13:26], axis=0)         [B, N, H]
  cat = concat([x, f1, f2], -1)             [B, N, 3H]
  h   = gelu(cat @ W1.T + b1)               W1 = nf4_dequant(codes1, scales1) [H, 3H]
  fg  = h @ W2.T + b2                       W2 = nf4_dequant(codes2, scales2) [H, H]
  out = w * LN(fg) + (1-w) * LN(x),         w = sigmoid(alpha)

Sharding: data-parallel over batch B=8 -> one batch element per NeuronCore.

Per-core plan (v3 -- chunked pipeline):
  - 6 token chunks of 128 (last chunk overlaps the previous by 39 tokens;
    identical values stored twice -- partial-partition DMA falls off the
    16-engine SDMA path and runs ~15x slower, so all tiles stay full-128).
  - The 26-layer `hidden` stream dominates HBM traffic (46 MB/core); it is
    issued as 12 large 3.8 MB DMAs on the sync HWDGE queue in chunk order so
    DMA stays saturated end-to-end.  Weights stream on the scalar queue
    behind chunk 0's loads.
  - layer sums entirely on DVE (GPSIMD port contention halves DVE throughput
    when both run -- measured), tree-shaped to amortize dispatch and release
    the hid tile early for DMA slot reuse.
  - cat^T is never materialized: GEMM1's k-loop reads x^T (host-transposed
    input), s1^T and s2^T (TensorE identity-transpose -> PSUM -> ACT copy)
    as three separate SBUF tiles.  No SBUF->SBUF xbar DMA at all.
  - GEMM1 weights-stationary -> h^T in PSUM; GELU(+b1 per-partition bias) on
    ACT -> g^T feeds GEMM2 as stationary; b2 is added by a rank-1 matmul
    (ones-row x b2-row) inside the accumulation group; ACT drains PSUM->fg
    while computing sum(fg) via accum_out.
  - LN stats: ACT accum_out gives S(v), S(v^2); DVE combines to mean/var,
    reciprocal+sqrt for rsqrt.  The gate combine uses 4x-mode tensor_scalar
    ops when the folded LN gains are feature-uniform (they are: ln gains are
    ones, biases zeros), falling back to scalar_tensor_tensor otherwise.

NF4 dequant of the (small, replicated) weights is host-side weight prep; the
bf16 weights are less DMA traffic than the int32 codes.
"""

import os
import sys

import numpy as np
import ml_dtypes

for _p in ("/opt/trn_rl_repo", "/root/.axon_site/_ro/trn_rl_repo"):
    if os.path.isdir(_p) and _p not in sys.path:
        sys.path.insert(0, _p)

import concourse.bass as bass
import concourse.mybir as mybir
import concourse.tile as tile
from concourse import bacc
from concourse import bass_utils

BF16 = mybir.dt.bfloat16
F32 = mybir.dt.float32
AF = mybir.ActivationFunctionType
ALU = mybir.AluOpType

NP_BF16 = ml_dtypes.bfloat16

P = 128
H = 1152
H3 = 3456
NT = 729          # tokens per core (N); B=8 cores
L = 26
KO1 = H3 // P     # 27 k-tiles for GEMM1
KO2 = H // P      # 9 k-tiles for GEMM2
MO = H // P       # 9 output-feature tiles
EPS = 1e-5
NCHUNK = 3        # fg free-dim chunks of 384
CH = H // NCHUNK  # 384

# Token chunks; the last starts at 601 so it is a full 128 tokens (tokens
# 601..639 are computed twice with identical values).
CHUNK_STARTS = [0, 128, 256, 384, 512, 601]
NCH = len(CHUNK_STARTS)

NF4_CODEBOOK = np.array([
    -1.0, -0.6961928009986877, -0.5250730514526367, -0.39491748809814453,
    -0.28444138169288635, -0.18477343022823334, -0.09105003625154495, 0.0,
    0.07958029955625534, 0.16093020141124725, 0.24611230194568634,
    0.33791524171829224, 0.4407098591327667, 0.5626170039176941,
    0.7229568362236023, 1.0], dtype=np.float32)

BLOCK = 64


def _dequant_nf4(codes, scales):
    """Match reference: codebook lookup * per-64-block absmax, cast bf16."""
    out_f, in_f = codes.shape
    w = NF4_CODEBOOK[codes].reshape(out_f, in_f // BLOCK, BLOCK)
    w = w * scales[:, :, None].astype(np.float32)
    return w.reshape(out_f, in_f)  # float32 (caller casts)


def _build_program(act=AF.Gelu, uniform_gate=True):
    nc = bacc.Bacc(
        "TRN2",
        target_bir_lowering=False,
        debug=False,
        num_devices=1,
    )
    x_d = nc.dram_tensor("x", (NT, H), BF16, kind="ExternalInput").ap()
    xtc_d = nc.dram_tensor("xtc", (NCH, P, MO, P), BF16, kind="ExternalInput").ap()
    hid_d = nc.dram_tensor("hid", (L, NT, H), BF16, kind="ExternalInput").ap()
    w1t_d = nc.dram_tensor("w1t", (H3, H), BF16, kind="ExternalInput").ap()
    w2t_d = nc.dram_tensor("w2t", (H, H), BF16, kind="ExternalInput").ap()
    b1s_d = nc.dram_tensor("b1s", (P, MO), F32, kind="ExternalInput").ap()
    b2s_d = nc.dram_tensor("b2s", (1, H), BF16, kind="ExternalInput").ap()
    ident_d = nc.dram_tensor("ident", (P, P), BF16, kind="ExternalInput").ap()
    # uniform path: gsc = per-partition (G1, G2) scalars; bcs = (Bc, 0)
    gsc_d = nc.dram_tensor("gsc", (P, 2), F32, kind="ExternalInput").ap()
    bcs_d = nc.dram_tensor("bcs", (P, 2), F32, kind="ExternalInput").ap()
    # general path: per-feature broadcasts
    g1b_d = nc.dram_tensor("g1b", (P, H), BF16, kind="ExternalInput").ap()
    g2b_d = nc.dram_tensor("g2b", (P, H), BF16, kind="ExternalInput").ap()
    bcb_d = nc.dram_tensor("bcb", (P, H), BF16, kind="ExternalInput").ap()
    out_d = nc.dram_tensor("out", (NT, H), BF16, kind="ExternalOutput").ap()

    with tile.TileContext(nc) as tc:
        _program(nc, tc, x_d, xtc_d, hid_d, w1t_d, w2t_d, b1s_d, b2s_d,
                 ident_d, gsc_d, bcs_d, g1b_d, g2b_d, bcb_d, out_d, act,
                 uniform_gate)

    nc.compile()
    return nc


def _program(nc, tc, x_d, xtc_d, hid_d, w1t_d, w2t_d, b1s_d, b2s_d, ident_d,
             gsc_d, bcs_d, g1b_d, g2b_d, bcb_d, out_d, act, uniform_gate):
    with (
        tc.tile_pool(name="consts", bufs=1) as cpool,
        tc.tile_pool(name="hid", bufs=4) as hpool,
        tc.tile_pool(name="xt", bufs=2) as xtpool,
        tc.tile_pool(name="x", bufs=3) as xpool,
        tc.tile_pool(name="scr", bufs=1) as scrpool,
        tc.tile_pool(name="acc", bufs=2) as apool,
        tc.tile_pool(name="st", bufs=2) as stpool,
        tc.tile_pool(name="g", bufs=2) as gpool,
        tc.tile_pool(name="fg", bufs=2) as fgpool,
        tc.tile_pool(name="tmp", bufs=2) as tpool,
        tc.tile_pool(name="dum", bufs=1) as dpool,
        tc.tile_pool(name="stats", bufs=2) as spool,
        tc.tile_pool(name="ps1", bufs=1, space="PSUM") as ps1pool,
        tc.tile_pool(name="ps2", bufs=3, space="PSUM") as ps2pool,
        tc.tile_pool(name="pt", bufs=2, space="PSUM") as ptpool,
    ):
        # ---- small constants first (sync queue; ~50 KB total) ----
        b1s_sb = cpool.tile([P, MO], F32)
        nc.sync.dma_start(b1s_sb, b1s_d)
        b2s_sb = cpool.tile([1, H], BF16)
        nc.sync.dma_start(b2s_sb, b2s_d)
        ident_sb = cpool.tile([P, P], BF16)
        nc.sync.dma_start(ident_sb, ident_d)
        if uniform_gate:
            gsc_sb = cpool.tile([P, 2], F32)
            nc.sync.dma_start(gsc_sb, gsc_d)
            bcs_sb = cpool.tile([P, 2], F32)
            nc.sync.dma_start(bcs_sb, bcs_d)
        else:
            g1b_sb = cpool.tile([P, H], BF16)
            nc.sync.dma_start(g1b_sb, g1b_d)
            g2b_sb = cpool.tile([P, H], BF16)
            nc.sync.dma_start(g2b_sb, g2b_d)
            bcb_sb = cpool.tile([P, H], BF16)
            nc.sync.dma_start(bcb_sb, bcb_d)
        ones_sb = cpool.tile([1, P], BF16)
        nc.vector.memset(ones_sb, 1.0)

        w1t_sb = cpool.tile([P, KO1, H], BF16)
        w2t_sb = cpool.tile([P, KO2, H], BF16)
        w1t_r = w1t_d.rearrange("(ko p) n -> p ko n", p=P)

        dummy = dpool.tile([P, H], BF16, tag="dummy")
        # DVE-serial scratch for the layer-sum trees (reused across chunks)
        scr = [scrpool.tile([P, 3, H], BF16, name=f"scr{i}", tag=f"scr{i}")
               for i in range(2)]

        def half_sum(h7, h6, dst, scr):
            """dst[t, f] = sum over the 7-layer and 6-layer pieces, on DVE.

            Tree-shaped to amortize DVE dispatch; each hid piece is fully
            consumed after two ops so its DMA slot recycles early."""
            t7 = scr[0]
            nc.vector.tensor_add(t7, h7[:, 0:3, :], h7[:, 3:6, :])
            nc.vector.tensor_add(t7[:, 2, :], t7[:, 2, :], h7[:, 6, :])
            t6 = scr[1]
            nc.vector.tensor_add(t6, h6[:, 0:3, :], h6[:, 3:6, :])
            nc.vector.tensor_add(t7, t7, t6)
            nc.vector.tensor_add(dst, t7[:, 0, :], t7[:, 1, :])
            nc.vector.tensor_add(dst, dst, t7[:, 2, :])

        def transpose_to(src, dst):
            """src [P, H] token-major -> dst [P, MO, P] feature-major."""
            for g0 in (0, 4, 8):
                g = min(4, MO - g0)
                pt = ptpool.tile([P, 4, P], BF16, tag="pt")
                for j in range(g):
                    nc.tensor.transpose(
                        pt[:, j, :],
                        src[:, (g0 + j) * P:(g0 + j + 1) * P],
                        ident_sb)
                nc.scalar.activation(dst[:, g0:g0 + g, :],
                                     pt[:, 0:g, :], AF.Copy)

        def make_epilogue(t0, xc, fg, sacc):
            """LN stats math + gate + store for a finished chunk.  Emitted
            one chunk late so the next chunk's layer-sum adds outrank it in
            the DVE priority queue (else TensorE's transposes stall)."""
            def epi():
                deriv = spool.tile([P, 8], F32, name="deriv", tag="deriv")
                nc.vector.tensor_add(sacc[:, 1:2], sacc[:, 4:5], sacc[:, 5:6])
                nc.vector.tensor_add(sacc[:, 1:2], sacc[:, 1:2], sacc[:, 6:7])
                # cols 0,1 = mean(x), mean(fg); 2,3 = E[v^2]+eps; 4,5 = mu^2
                nc.vector.tensor_scalar_mul(deriv[:, 0:2], sacc[:, 0:2],
                                            1.0 / H)
                nc.vector.tensor_scalar(deriv[:, 2:4], sacc[:, 2:4],
                                        1.0 / H, EPS, ALU.mult, ALU.add)
                nc.vector.tensor_tensor(deriv[:, 4:6], deriv[:, 0:2],
                                        deriv[:, 0:2], ALU.mult)
                nc.vector.tensor_tensor(deriv[:, 6:8], deriv[:, 2:4],
                                        deriv[:, 4:6], ALU.subtract)
                igt = spool.tile([P, 2], F32, name="igt", tag="ig")
                nc.vector.reciprocal(igt, deriv[:, 6:8])
                nc.scalar.activation(igt, igt, AF.Sqrt)

                tmp1 = tpool.tile([P, H], BF16, name="tmp1", tag="tmp1")
                if uniform_gate:
                    # acol = (G1*ig1, G2*ig2); Bc via bcs (always 0 here)
                    acol = spool.tile([P, 2], F32, name="acol", tag="acol")
                    nc.vector.tensor_tensor(acol, igt, gsc_sb, ALU.mult)
                    nc.vector.tensor_scalar(tmp1, xc, deriv[:, 0:1],
                                            acol[:, 0:1], ALU.subtract,
                                            ALU.mult)
                    nc.vector.tensor_scalar(fg, fg, deriv[:, 1:2],
                                            acol[:, 1:2], ALU.subtract,
                                            ALU.mult)
                    nc.vector.scalar_tensor_tensor(
                        tmp1, tmp1, bcs_sb[:, 0:1], fg, ALU.add, ALU.add)
                else:
                    nc.vector.scalar_tensor_tensor(
                        tmp1, xc, deriv[:, 0:1], g1b_sb,
                        ALU.subtract, ALU.mult)
                    nc.vector.scalar_tensor_tensor(
                        fg, fg, deriv[:, 1:2], g2b_sb,
                        ALU.subtract, ALU.mult)
                    nc.vector.scalar_tensor_tensor(
                        tmp1, tmp1, igt[:, 0:1], bcb_sb,
                        ALU.mult, ALU.add)
                    nc.vector.scalar_tensor_tensor(
                        tmp1, fg, igt[:, 1:2], tmp1,
                        ALU.mult, ALU.add)
                nc.scalar.dma_start(out_d[t0:t0 + P, :], tmp1)
            return epi

        pending_epi = None
        for c, t0 in enumerate(CHUNK_STARTS):
            # ---- DMA issues (loads only; stores go at the chunk end) ----
            hps = []
            for l0, nl in ((0, 7), (7, 6), (13, 7), (20, 6)):
                hp = hpool.tile([P, 7, H], BF16, tag="hid")
                nc.sync.dma_start(
                    hp[:, 0:nl, :],
                    hid_d[l0:l0 + nl, t0:t0 + P, :].rearrange(
                        "l p f -> p l f"))
                hps.append(hp)
            xt = xtpool.tile([P, MO, P], BF16, tag="xtc")
            nc.scalar.dma_start(xt, xtc_d[c])
            if c == 0:
                # weights stream behind chunk 0's x^T on the scalar queue,
                # ordered so GEMM1's k-outer loop can start early
                nc.scalar.dma_start(w1t_sb[:, 0:9, :], w1t_r[:, 0:9, :])
            xc = xpool.tile([P, H], BF16, tag="x")
            nc.scalar.dma_start(xc, x_d[t0:t0 + P, :])
            if c == 0:
                nc.scalar.dma_start(w1t_sb[:, 9:18, :], w1t_r[:, 9:18, :])
                nc.scalar.dma_start(w1t_sb[:, 18:27, :], w1t_r[:, 18:27, :])
                nc.scalar.dma_start(
                    w2t_sb, w2t_d.rearrange("(ko p) n -> p ko n", p=P))

            # ---- 13-layer sums on DVE ----
            s1 = apool.tile([P, H], BF16, tag="s1")
            half_sum(hps[0], hps[1], s1, scr)
            s2 = apool.tile([P, H], BF16, tag="s2")
            half_sum(hps[2], hps[3], s2, scr)

            sacc = spool.tile([P, 8], F32, tag="sacc")

            # ---- GEMM1 (weights-stationary, k-outer) with s1/s2 transposes
            # (TensorE identity -> PSUM -> ACT copy) issued ahead of the
            # k-groups that consume them, so the ACT copies hide under the
            # preceding matmuls.  The final k-group is m-outer so each GELU
            # fires as soon as its m-tile finishes.
            ps1 = ps1pool.tile([P, MO, P], F32, tag="ps1")
            s1T = stpool.tile([P, MO, P], BF16, tag="s1T")
            s2T = stpool.tile([P, MO, P], BF16, tag="s2T")
            gT = gpool.tile([P, MO, P], BF16, tag="gT")

            def mm1(kk, mm, rhs):
                # start=True marks the whole 2KB PSUM bank pending-zero, so
                # only the first matmul touching each bank sets it; the
                # other m-slices' first writes land on still-pending bytes
                # and overwrite (HW has_written semantics; sim mirrors it).
                nc.tensor.matmul(
                    ps1[:, mm, :],
                    lhsT=w1t_sb[:, kk, mm * P:(mm + 1) * P],
                    rhs=rhs,
                    start=(kk == 0 and mm % 4 == 0),
                    stop=(kk == KO1 - 1),
                    skip_group_check=True,
                )

            transpose_to(s1, s1T)
            for kk in range(0, MO):
                for mm in range(MO):
                    mm1(kk, mm, xt[:, kk, :])
            transpose_to(s2, s2T)
            for kk in range(MO, 2 * MO):
                for mm in range(MO):
                    mm1(kk, mm, s1T[:, kk - MO, :])
            for mm in range(MO):
                for kk in range(2 * MO, 3 * MO):
                    mm1(kk, mm, s2T[:, kk - 2 * MO, :])
                nc.scalar.activation(gT[:, mm, :], ps1[:, mm, :], act,
                                     bias=b1s_sb[:, mm:mm + 1])

            # ---- previous chunk's stats/gate/store (deferred: see above) --
            if pending_epi is not None:
                pending_epi()

            # ---- LN1(x) raw sums on ACT (fill the GELU->drain gap) ----
            nc.scalar.activation(dummy, xc, AF.Copy,
                                 accum_out=sacc[:, 0:1])
            nc.scalar.activation(dummy, xc, AF.Square,
                                 accum_out=sacc[:, 2:3])

            # ---- GEMM2 (g^T-stationary, k-outer) + b2 rank-1 + ACT drain --
            fg = fgpool.tile([P, H], BF16, tag="fg")
            ps2s = [ps2pool.tile([P, CH], F32, name=f"ps2_{nn}", tag="ps2")
                    for nn in range(NCHUNK)]
            for kk in range(KO2):
                for nn in range(NCHUNK):
                    nc.tensor.matmul(
                        ps2s[nn],
                        lhsT=gT[:, kk, :],
                        rhs=w2t_sb[:, kk, nn * CH:(nn + 1) * CH],
                        start=(kk == 0),
                        stop=False,
                    )
            for nn in range(NCHUNK):
                nc.tensor.matmul(
                    ps2s[nn],
                    lhsT=ones_sb,
                    rhs=b2s_sb[0:1, nn * CH:(nn + 1) * CH],
                    start=False,
                    stop=True,
                )
                nc.scalar.activation(fg[:, nn * CH:(nn + 1) * CH],
                                     ps2s[nn], AF.Copy,
                                     accum_out=sacc[:, 4 + nn:5 + nn])
            nc.scalar.activation(dummy, fg, AF.Square,
                                 accum_out=sacc[:, 3:4])

            pending_epi = make_epilogue(t0, xc, fg, sacc)
        pending_epi()


_NC_CACHE = {}


def _get_nc(uniform_gate=True):
    key = ("nc", uniform_gate)
    if key not in _NC_CACHE:
        _NC_CACHE[key] = _build_program(uniform_gate=uniform_gate)
    return _NC_CACHE[key]


def _host_prep(codes1, scales1, b1, codes2, scales2, b2,
               ln1_g, ln1_b, ln2_g, ln2_b, alpha):
    # W1 with 1/13 folded into the f1/f2 column blocks (mean -> sum)
    w1 = _dequant_nf4(codes1, scales1)
    # match reference rounding: dequant result is cast to bf16 first
    w1 = w1.astype(NP_BF16).astype(np.float32)
    w1[:, H:] *= np.float32(1.0 / 13.0)
    w1t = np.ascontiguousarray(w1.T).astype(NP_BF16)

    w2 = _dequant_nf4(codes2, scales2).astype(NP_BF16)
    w2t = np.ascontiguousarray(w2.astype(np.float32).T).astype(NP_BF16)

    b1s = np.ascontiguousarray(
        b1.astype(np.float32).reshape(MO, P).T)  # [P, MO]
    b2s = np.ascontiguousarray(b2.astype(NP_BF16).reshape(1, H))

    ident = np.eye(P, dtype=NP_BF16)

    a32 = alpha.astype(np.float32)
    w_gate = (1.0 / (1.0 + np.exp(-a32[0]))).astype(NP_BF16)
    one_minus = (NP_BF16(1.0) - w_gate)
    g1 = (one_minus.astype(np.float32) * ln1_g.astype(np.float32))
    g2 = (w_gate.astype(np.float32) * ln2_g.astype(np.float32))
    bc = (w_gate.astype(np.float32) * ln2_b.astype(np.float32)
          + one_minus.astype(np.float32) * ln1_b.astype(np.float32))

    uniform = (np.ptp(g1) == 0.0 and np.ptp(g2) == 0.0 and np.all(bc == 0.0))
    gsc = np.ascontiguousarray(
        np.broadcast_to(np.array([g1[0], g2[0]], np.float32), (P, 2)))
    bcs = np.zeros((P, 2), np.float32)

    g1b = np.ascontiguousarray(np.broadcast_to(g1.astype(NP_BF16), (P, H)))
    g2b = np.ascontiguousarray(np.broadcast_to(g2.astype(NP_BF16), (P, H)))
    bcb = np.ascontiguousarray(np.broadcast_to(bc.astype(NP_BF16), (P, H)))
    return w1t, w2t, b1s, b2s, ident, gsc, bcs, g1b, g2b, bcb, uniform


def _xtc_prep(x):
    """[729, H] token-major -> [NCH, P, MO, P] feature-major token chunks."""
    xT = np.ascontiguousarray(x.T).reshape(MO, P, NT)
    out = np.empty((NCH, P, MO, P), dtype=NP_BF16)
    for c, t0 in enumerate(CHUNK_STARTS):
        out[c] = xT[:, :, t0:t0 + P].transpose(1, 0, 2)
    return out


def make_in_maps(image_features, hidden, codes1, scales1, b1, codes2, scales2,
                 b2, ln1_g, ln1_b, ln2_g, ln2_b, alpha):
    (w1t, w2t, b1s, b2s, ident, gsc, bcs, g1b, g2b, bcb,
     uniform) = _host_prep(codes1, scales1, b1, codes2, scales2, b2,
                           ln1_g, ln1_b, ln2_g, ln2_b, alpha)
    B = image_features.shape[0]
    in_maps = []
    for c in range(B):
        xc = np.ascontiguousarray(image_features[c]).astype(NP_BF16, copy=False)
        in_maps.append({
            "x": xc,
            "xtc": _xtc_prep(xc),
            "hid": np.ascontiguousarray(hidden[:, c]).astype(NP_BF16, copy=False),
            "w1t": w1t, "w2t": w2t, "b1s": b1s, "b2s": b2s, "ident": ident,
            "gsc": gsc, "bcs": bcs,
            "g1b": g1b, "g2b": g2b, "bcb": bcb,
        })
    return in_maps, uniform


def kernel(image_features, hidden, codes1, scales1, b1, codes2, scales2, b2,
           ln1_g, ln1_b, ln2_g, ln2_b, alpha, _trace=False):
    B, N, Hin = image_features.shape
    assert (B, N, Hin) == (8, NT, H), (B, N, Hin)
    in_maps, uniform = make_in_maps(
        image_features, hidden, codes1, scales1, b1, codes2, scales2, b2,
        ln1_g, ln1_b, ln2_g, ln2_b, alpha)
    nc = _get_nc(uniform_gate=uniform)
    res = bass_utils.run_bass_kernel_spmd(
        nc, in_maps, core_ids=list(range(8)), trace=_trace)
    out = np.stack([res.results[c]["out"] for c in range(8)])
    if _trace:
        kernel._last_results = res
    return out.astype(image_features.dtype, copy=False)
